# revision 9
# baseline (speedup 1.0000x reference)
"""BiLSTM tagger Trainium kernel — 8-core SPMD, data-parallel over
(direction, batch-quarter).

Core i: direction d=i//4 (0=fwd, 1=bwd), batch quarter q=i%4 (rows 16q..16q+16).
All cores run an IDENTICAL program; per-core specialization enters via input
data (bwd cores get time-reversed token indices + their direction's weights).

Transposed recurrence layout (v2): all recurrence tensors are h-major:
  psum gates  [128 q, 256]  col = 16*Bk + b,  Bk = g'*4 + j  (g' in i,f,o,g order)
  c, h        [128 q, 64]   col = 16*j + b   (h feature index = 128*j + q)
The cell output h is directly the moving operand of the next step's matmuls
(lhsT = Whh blocks stationary, rhs = h slice [128,16]) -> no per-step
transpose, and the PE streams only ~1.3K rows/step instead of ~20K.

Pipeline per core:
  A. embedding gather (indirect DMA) + PE transpose -> X^T (f32r);
     z1^T = Wih1 @ X^T + b1 -> DRAM bf16 [T,128,256] (step-t slabs)
  B. L1 recurrence, T steps (weight-stationary, single chain)
  C. pairwise AllGather(rev hist) -> peer-direction features; z2^T proj
  D. L2 recurrence   E. pairwise AllGather + FC -> logits [T*16, 64] f32

Host assembles: cores 0..3 give forward-time logits for quarters 0..3.
"""
import numpy as np
import ml_dtypes

import concourse.bacc as bacc
import concourse.bass as bass
import concourse.mybir as mybir
import concourse.tile as tile
from concourse.bass_utils import run_bass_kernel_spmd

F32 = mybir.dt.float32
F32R = mybir.dt.float32r
BF16 = mybir.dt.bfloat16
I32 = mybir.dt.int32
AF = mybir.ActivationFunctionType
BF16_NP = ml_dtypes.bfloat16

B, V, E, H, TAGS = 64, 50000, 512, 512, 50
NCORES = 8
BQ = B // 4  # 16: batch rows per core


def _build(T, passes=1):
    G = T * BQ // 128          # embedding row-groups of 128
    N16 = T * BQ               # history cols per h-slice
    CS = min(512, N16)         # projection N-chunk (psum-bank limited)
    NCH = N16 // CS
    TPC = CS // BQ             # timesteps covered by one projection chunk
    nc = bacc.Bacc("TRN2", target_bir_lowering=False, debug=False,
                   num_devices=NCORES)

    emb_d = nc.dram_tensor("emb", [V, E], F32, kind="ExternalInput").ap()
    idx_d = nc.dram_tensor("idx", [128, G], I32, kind="ExternalInput").ap()
    idxh_d = nc.dram_tensor("idxh", [128, 4], I32, kind="ExternalInput").ap()
    w1_d = nc.dram_tensor("w1", [4, 128, 2048], F32R, kind="ExternalInput").ap()
    b1_d = nc.dram_tensor("b1", [1, 2048], F32R, kind="ExternalInput").ap()
    wm1_d = nc.dram_tensor("wm1", [4, 128, 2048], BF16, kind="ExternalInput").ap()
    w2_d = nc.dram_tensor("w2", [8, 128, 2048], BF16, kind="ExternalInput").ap()
    b2_d = nc.dram_tensor("b2", [1, 2048], BF16, kind="ExternalInput").ap()
    wm2_d = nc.dram_tensor("wm2", [4, 128, 2048], BF16, kind="ExternalInput").ap()
    fcw_d = nc.dram_tensor("fcw", [8, 128, 64], BF16, kind="ExternalInput").ap()
    fcb_d = nc.dram_tensor("fcb", [1, 64], BF16, kind="ExternalInput").ap()
    ones_d = nc.dram_tensor("ones", [1, 512], F32R, kind="ExternalInput").ap()
    onesb_d = nc.dram_tensor("onesb", [1, 512], BF16, kind="ExternalInput").ap()
    id128_d = nc.dram_tensor("id128", [128, 128], F32, kind="ExternalInput").ap()
    idbf_d = nc.dram_tensor("idbf", [128, 128], BF16, kind="ExternalInput").ap()
    logits_d = nc.dram_tensor("logits", [N16, 64], F32,
                              kind="ExternalOutput").ap()

    with tile.TileContext(nc) as tc:
        with tc.tile_pool(name="pconst", bufs=1) as pconst, \
             tc.tile_pool(name="pdram", bufs=1, space="DRAM") as pdram:
            ones = pconst.tile([1, 512], F32R, name="ones")
            onesb = pconst.tile([1, 512], BF16, name="onesb")
            id128 = pconst.tile([128, 128], F32, name="id128")
            idbf = pconst.tile([128, 128], BF16, name="idbf")
            idxs = pconst.tile([128, G], I32, name="idxs")
            idxh = pconst.tile([128, 4], I32, name="idxh")
            b1r = pconst.tile([1, 2048], F32R, name="b1r")
            b2r = pconst.tile([1, 2048], BF16, name="b2r")
            fcb = pconst.tile([1, 64], BF16, name="fcb")
            nc.sync.dma_start(ones[:], ones_d[:])
            nc.sync.dma_start(onesb[:], onesb_d[:])
            nc.sync.dma_start(id128[:], id128_d[:])
            nc.sync.dma_start(idbf[:], idbf_d[:])
            nc.sync.dma_start(idxs[:], idx_d[:])
            nc.sync.dma_start(idxh[:], idxh_d[:])
            nc.sync.dma_start(b1r[:], b1_d[:])
            nc.sync.dma_start(b2r[:], b2_d[:])
            nc.sync.dma_start(fcb[:], fcb_d[:])

            z1T = pdram.tile([T, 128, 256], BF16, name="z1T")
            z2T = pdram.tile([T, 128, 256], BF16, name="z2T")
            rev1 = pdram.tile([4, 128, N16], BF16, name="rev1")
            rev2 = pdram.tile([4, 128, N16], BF16, name="rev2")
            ag1 = pdram.tile([1024, N16], BF16, name="ag1")
            ag2 = pdram.tile([1024, N16], BF16, name="ag2")
            AG_GROUPS = [[0, 4], [1, 5], [2, 6], [3, 7]]

            # z^T projection sweep: out z_T[t, q, 16*Bk+b] from K k-slices.
            def proj_zT(zT_dram, wsb, bsb, nk, rhs_fn, pool, psum_pool):
                for Bk in range(16):
                    for n in range(NCH):
                        pg = psum_pool.tile([128, CS], F32, tag="pgz")
                        nc.tensor.matmul(pg[:], lhsT=bsb[:, 128 * Bk:128 * Bk + 128],
                                         rhs=(ones if nk == 4 else onesb)[:, 0:CS],
                                         start=True, stop=False)
                        for k in range(nk):
                            nc.tensor.matmul(
                                pg[:], lhsT=wsb(k)[:, 128 * Bk:128 * Bk + 128],
                                rhs=rhs_fn(k, n),
                                start=False, stop=(k == nk - 1))
                        zst = pool.tile([128, CS], BF16, tag="zst", bufs=4)
                        if (Bk + n) % 2 == 0:
                            nc.vector.tensor_copy(zst[:], pg[:])
                        else:
                            nc.scalar.activation(zst[:], pg[:], AF.Copy)
                        nc.sync.dma_start(
                            zT_dram[TPC * n:TPC * (n + 1), :,
                                    16 * Bk:16 * Bk + 16]
                            .rearrange("t p b -> p t b"),
                            zst[:].rearrange("p (t b) -> p t b", b=BQ))

            # recurrence: z placement (identity mm) + weight-stationary h mms.
            # mms are gate-grouped (g,i,f,o) with per-region stop so the ACT
            # ops start while the PE drain continues.
            def recurrence(zT_dram, wm_sb, hist, rev_dram, mid_hook=None):
                with tc.tile_pool(name="pR", bufs=1) as pR, \
                     tc.tile_pool(name="psR", bufs=1, space="PSUM") as psR:
                    nc.vector.memset(hist[:, 0:64], 0.0)
                    c_prev = pR.tile([128, 64], F32, tag="c", bufs=2)
                    nc.vector.memset(c_prev[:], 0.0)
                    for t in range(T):
                        zs = pR.tile([128, 256], BF16, tag="z", bufs=8)
                        nc.sync.dma_start(zs[:], zT_dram[t])
                        pg = psR.tile([128, 256], F32, tag="pgR", bufs=2)
                        nc.tensor.matmul(pg[:], lhsT=idbf[:], rhs=zs[:],
                                         start=True, stop=False,
                                         skip_group_check=True)
                        h_prev = hist[:, 64 * t:64 * t + 64]

                        def mm_group(gp):
                            for Bk in range(4 * gp, 4 * gp + 4):
                                for k in range(4):
                                    nc.tensor.matmul(
                                        pg[:, 16 * Bk:16 * Bk + 16],
                                        lhsT=wm_sb[k][:, 128 * Bk:128 * Bk + 128],
                                        rhs=h_prev[:, 16 * k:16 * k + 16],
                                        start=False, stop=(k == 3),
                                        skip_group_check=True)
                        s = pR.tile([128, 192], F32, tag="s", bufs=2)
                        tg = pR.tile([128, 64], F32, tag="tg", bufs=2)
                        mm_group(3)                                   # g
                        nc.scalar.activation(tg[:], pg[:, 192:256], AF.Tanh)
                        mm_group(0)                                   # i
                        mm_group(1)                                   # f
                        nc.scalar.activation(s[:, 0:128], pg[:, 0:128],
                                             AF.Sigmoid)
                        mm_group(2)                                   # o
                        nc.scalar.activation(s[:, 128:192], pg[:, 128:192],
                                             AF.Sigmoid)
                        tmp1 = pR.tile([128, 64], F32, tag="tmp1", bufs=2)
                        tmp2 = pR.tile([128, 64], F32, tag="tmp2", bufs=2)
                        c_new = pR.tile([128, 64], F32, tag="c", bufs=2)
                        nc.vector.tensor_mul(tmp1[:], s[:, 64:128], c_prev[:])
                        nc.gpsimd.tensor_mul(tmp2[:], s[:, 0:64], tg[:])
                        nc.vector.tensor_add(c_new[:], tmp1[:], tmp2[:])
                        tc_t = pR.tile([128, 64], F32, tag="tc", bufs=2)
                        nc.scalar.activation(tc_t[:], c_new[:], AF.Tanh)
                        h_new = hist[:, 64 * (t + 1):64 * (t + 2)]
                        nc.vector.tensor_mul(h_new, s[:, 128:192], tc_t[:])
                        tr = T - 1 - t
                        nc.sync.dma_start(
                            rev_dram[:, :, BQ * tr:BQ * tr + BQ]
                            .rearrange("k p b -> p k b"),
                            h_new.rearrange("p (k b) -> p k b", b=BQ))
                        c_prev = c_new
                        if mid_hook is not None and t == T // 2:
                            mid_hook()

            for _pass in range(passes):
                # ---------- phase A: embed + z1^T projection ----------
                with tc.tile_pool(name="pA", bufs=1) as pA, \
                     tc.tile_pool(name="psA", bufs=4, space="PSUM") as psA, \
                     tc.tile_pool(name="psAt", bufs=4, space="PSUM") as psAt:
                    w1sb = [pA.tile([128, 2048], F32R, name=f"w1sb{k}")
                            for k in range(4)]
                    for k in range(4):
                        nc.sync.dma_start(w1sb[k][:], w1_d[k])
                    xT = [pA.tile([128, G * 128], F32R, name=f"xT{k}")
                          for k in range(4)]
                    for g in range(G):
                        es = pA.tile([128, 512], F32, tag="es", bufs=3)
                        nc.gpsimd.indirect_dma_start(
                            out=es[:], out_offset=None, in_=emb_d[:],
                            in_offset=bass.IndirectOffsetOnAxis(
                                ap=idxs[:, g:g + 1], axis=0))
                        for k in range(4):
                            pt = psAt.tile([128, 128], F32, tag="ptA")
                            nc.tensor.transpose(
                                pt[:], es[:, 128 * k:128 * (k + 1)], id128[:])
                            if k % 2 == 0:
                                nc.vector.tensor_copy(
                                    xT[k][:, 128 * g:128 * (g + 1)], pt[:])
                            else:
                                nc.scalar.activation(
                                    xT[k][:, 128 * g:128 * (g + 1)], pt[:],
                                    AF.Copy)
                    proj_zT(z1T, lambda k: w1sb[k], b1r, 4,
                            lambda k, n: xT[k][:, CS * n:CS * (n + 1)],
                            pA, psA)

                # ---------- phase B: L1 recurrence ----------
                with tc.tile_pool(name="pH1", bufs=1) as pH1:
                    hist1 = pH1.tile([128, (T + 1) * 64], BF16, name="hist1")
                    with tc.tile_pool(name="pB", bufs=1) as pB:
                        wm1sb = [pB.tile([128, 2048], BF16, name=f"wm1sb{k}")
                                 for k in range(4)]
                        for k in range(4):
                            nc.sync.dma_start(wm1sb[k][:], wm1_d[k])
                        def ag1_late():
                            nc.gpsimd.collective_compute(
                                "AllGather", mybir.AluOpType.bypass,
                                replica_groups=AG_GROUPS,
                                ins=[rev1[:, :, N16 // 2:]
                                     .rearrange("k p t -> (k p) t")],
                                outs=[ag1[:, N16 // 2:]])
                        recurrence(z1T, wm1sb, hist1, rev1, mid_hook=ag1_late)

                    # ---------- phase C: AG1 + peer gather + z2^T ----------
                    nc.gpsimd.collective_compute(
                        "AllGather", mybir.AluOpType.bypass,
                        replica_groups=AG_GROUPS,
                        ins=[rev1[:, :, 0:N16 // 2]
                             .rearrange("k p t -> (k p) t")],
                        outs=[ag1[:, 0:N16 // 2]])
                    with tc.tile_pool(name="pC", bufs=1) as pC, \
                         tc.tile_pool(name="psC", bufs=4, space="PSUM") as psC:
                        xp = [pC.tile([128, N16], BF16, name=f"xp_{k}")
                              for k in range(4)]
                        for k in range(4):
                            nc.gpsimd.indirect_dma_start(
                                out=xp[k][:], out_offset=None, in_=ag1[:],
                                in_offset=bass.IndirectOffsetOnAxis(
                                    ap=idxh[:, k:k + 1], axis=0))
                        w2sb = [pC.tile([128, 2048], BF16, name=f"w2sb{k}")
                                for k in range(8)]
                        for k in range(8):
                            nc.sync.dma_start(w2sb[k][:], w2_d[k])
                        hk = [pC.tile([128, N16], BF16, name=f"hk_{k}")
                              for k in range(4)]
                        hv = hist1[:, 64:].rearrange(
                            "p (t k b) -> p k t b", k=4, b=BQ)
                        for k in range(4):
                            nc.vector.tensor_copy(
                                hk[k][:].rearrange("p (t b) -> p t b", b=BQ),
                                hv[:, k])

                        def rhs2(k, n):
                            if k < 4:
                                return hk[k][:, CS * n:CS * (n + 1)]
                            return xp[k - 4][:, CS * n:CS * (n + 1)]
                        proj_zT(z2T, lambda k: w2sb[k], b2r, 8, rhs2, pC, psC)

                # ---------- phase D: L2 recurrence ----------
                with tc.tile_pool(name="pH2", bufs=1) as pH2:
                    hist2 = pH2.tile([128, (T + 1) * 64], BF16, name="hist2")
                    with tc.tile_pool(name="pD", bufs=1) as pD:
                        wm2sb = [pD.tile([128, 2048], BF16, name=f"wm2sb{k}")
                                 for k in range(4)]
                        for k in range(4):
                            nc.sync.dma_start(wm2sb[k][:], wm2_d[k])
                        def ag2_late():
                            nc.gpsimd.collective_compute(
                                "AllGather", mybir.AluOpType.bypass,
                                replica_groups=AG_GROUPS,
                                ins=[rev2[:, :, N16 // 2:]
                                     .rearrange("k p t -> (k p) t")],
                                outs=[ag2[:, N16 // 2:]])
                        recurrence(z2T, wm2sb, hist2, rev2, mid_hook=ag2_late)

                    # ---------- phase E: AG2 + peer gather + FC ----------
                    nc.gpsimd.collective_compute(
                        "AllGather", mybir.AluOpType.bypass,
                        replica_groups=AG_GROUPS,
                        ins=[rev2[:, :, 0:N16 // 2]
                             .rearrange("k p t -> (k p) t")],
                        outs=[ag2[:, 0:N16 // 2]])
                    with tc.tile_pool(name="pE", bufs=1) as pE, \
                         tc.tile_pool(name="psE", bufs=4, space="PSUM") as psE:
                        xp2 = [pE.tile([128, N16], BF16, name=f"xp2_{k}")
                               for k in range(4)]
                        for k in range(4):
                            nc.gpsimd.indirect_dma_start(
                                out=xp2[k][:], out_offset=None, in_=ag2[:],
                                in_offset=bass.IndirectOffsetOnAxis(
                                    ap=idxh[:, k:k + 1], axis=0))
                        fcw = [pE.tile([128, 64], BF16, name=f"fcwsb{k}")
                               for k in range(8)]
                        for k in range(8):
                            nc.sync.dma_start(fcw[k][:], fcw_d[k])
                        hk2 = [pE.tile([128, N16], BF16, name=f"hk2_{k}")
                               for k in range(4)]
                        hv2 = hist2[:, 64:].rearrange(
                            "p (t k b) -> p k t b", k=4, b=BQ)
                        for k in range(4):
                            nc.vector.tensor_copy(
                                hk2[k][:].rearrange("p (t b) -> p t b", b=BQ),
                                hv2[:, k])
                        for m in range(N16 // 128):
                            pg = psE.tile([128, 64], F32, tag="pgE")
                            nc.tensor.matmul(pg[:], lhsT=onesb[:, 0:128],
                                             rhs=fcb[:],
                                             start=True, stop=False)
                            for k in range(8):
                                st = (hk2[k][:, 128 * m:128 * (m + 1)]
                                      if k < 4 else
                                      xp2[k - 4][:, 128 * m:128 * (m + 1)])
                                nc.tensor.matmul(
                                    pg[:], lhsT=st, rhs=fcw[k][:, :],
                                    start=False, stop=(k == 7))
                            lst = pE.tile([128, 64], F32, tag="lst", bufs=3)
                            nc.vector.tensor_copy(lst[:], pg[:])
                            nc.sync.dma_start(
                                logits_d[128 * m:128 * (m + 1), :], lst[:])

    nc.compile()
    return nc


# ---------------- host-side data prep ----------------

GPERM = [0, 1, 3, 2]  # device gate order: i, f, o, g (torch: i, f, g, o)


def _wT(w):
    # w: [2048(torch gates), D] -> [D//128, 128, 2048] lhsT blocks:
    # out[k][p, 512g' + 128j + q] = w[512*GPERM[g'] + 128j + q, 128k + p]
    D = w.shape[1]
    w5 = w.reshape(4, 4, 128, D // 128, 128)[GPERM]   # [g', j, q, k, p]
    return np.ascontiguousarray(np.transpose(w5, (3, 4, 0, 1, 2))
                                .reshape(D // 128, 128, 2048))


def _bT(b):
    return b.reshape(4, 4, 128)[GPERM].reshape(1, 2048)


_CACHE = {}


def kernel(x, lengths, emb,
           Wih_f1, Whh_f1, bih_f1, bhh_f1,
           Wih_b1, Whh_b1, bih_b1, bhh_b1,
           Wih_f2, Whh_f2, bih_f2, bhh_f2,
           Wih_b2, Whh_b2, bih_b2, bhh_b2,
           fc_W, fc_b, _T=None, _passes=1):
    x = np.asarray(x)
    T = x.shape[1] if _T is None else _T
    G = T * BQ // 128

    key = (T, _passes)
    if key not in _CACHE:
        _CACHE[key] = _build(T, _passes)
    nc = _CACHE[key]

    emb = np.asarray(emb, np.float32)
    f32 = lambda a: np.asarray(a, np.float32)
    layers = {
        0: (f32(Wih_f1), f32(Whh_f1), f32(bih_f1) + f32(bhh_f1),
            f32(Wih_f2), f32(Whh_f2), f32(bih_f2) + f32(bhh_f2)),
        1: (f32(Wih_b1), f32(Whh_b1), f32(bih_b1) + f32(bhh_b1),
            f32(Wih_b2), f32(Whh_b2), f32(bih_b2) + f32(bhh_b2)),
    }
    fc_W = f32(fc_W)
    fc_b = f32(fc_b)

    common = {
        "emb": emb,
        "ones": np.ones((1, 512), np.float32),
        "onesb": np.ones((1, 512), BF16_NP),
        "id128": np.eye(128, dtype=np.float32),
        "idbf": np.eye(128, dtype=np.float32).astype(BF16_NP),
        "fcb": np.pad(fc_b, (0, 14)).reshape(1, 64).astype(BF16_NP),
    }

    in_maps = []
    for i in range(NCORES):
        d, q = i // 4, i % 4
        wih1, whh1, bsum1, wih2, whh2, bsum2 = layers[d]
        xq = np.asarray(x[BQ * q:BQ * (q + 1), :T], np.int32)
        if d == 1:
            xq = xq[:, ::-1]
        # idx[p, g]: row r = 128g + p = 16t + b -> token xq[b, t]
        rr = np.arange(T * BQ)
        tt, bb = rr // BQ, rr % BQ
        idx_np = xq[bb, tt].reshape(G, 128).T.astype(np.int32).copy()
        # peer block in pairwise AG output: [fwd(512), bwd(512)]
        peer0 = 512 * (1 - d)
        pp = np.arange(128)
        idxh_np = (peer0 + 128 * np.arange(4)[None, :] + pp[:, None]
                   ).astype(np.int32)
        # layer-2 input feature order: own direction first, then peer
        own_sl = slice(512 * d, 512 * (d + 1))
        peer_sl = slice(512 * (1 - d), 512 * (2 - d))
        w2eff = np.concatenate([wih2[:, own_sl], wih2[:, peer_sl]], axis=1)
        fceff = np.concatenate([fc_W[:, own_sl], fc_W[:, peer_sl]], axis=1)
        fcmov = np.zeros((8, 128, 64), BF16_NP)
        for k in range(8):
            fcmov[k, :, :TAGS] = fceff[:, 128 * k:128 * (k + 1)].T
        in_maps.append(dict(
            common,
            idx=idx_np,
            idxh=idxh_np,
            w1=_wT(wih1).astype(np.float32),
            b1=_bT(bsum1).astype(np.float32),
            wm1=_wT(whh1).astype(BF16_NP),
            w2=_wT(w2eff).astype(BF16_NP),
            b2=_bT(bsum2).astype(BF16_NP),
            wm2=_wT(whh2).astype(BF16_NP),
            fcw=fcmov,
        ))

    res = run_bass_kernel_spmd(nc, in_maps, core_ids=list(range(NCORES)))

    out = np.zeros((B, T, TAGS), np.float32)
    for q in range(4):
        lg = res.results[q]["logits"][:, :TAGS]
        out[BQ * q:BQ * (q + 1)] = lg.reshape(T, BQ, TAGS).transpose(1, 0, 2)
    return out


# revision 28
# speedup vs baseline: 9462.4446x; 9462.4446x over previous
"""BiLSTM tagger Trainium kernel — 8-core SPMD, data-parallel over
(direction, batch-quarter).

Core i: direction d=i//4 (0=fwd, 1=bwd), batch quarter q=i%4 (rows 16q..16q+16).
All cores run an IDENTICAL program; per-core specialization enters via input
data (bwd cores get time-reversed token indices + their direction's weights).

Transposed recurrence layout: all recurrence tensors are h-major:
  psum gates  [128 q, 256]  col = 16*Bk + b,  Bk = g'*4 + j  (g' in i,f,o,g order)
  c, h        [128 q, 64]   col = 16*j + b   (h feature index = 128*j + q)
The cell output h is directly the moving operand of the next step's matmuls
(lhsT = Whh blocks stationary, rhs = h slice [128,16]) -> no per-step
transpose; the PE streams only ~1.3K rows/step.

Software pipelining: the z projections never touch DRAM — each projection
chunk (512 batch-time rows x 2048 gates) is computed into an SBUF staging
buffer (zbuf) interleaved between the recurrence steps of the previous
chunk.  Layer 1 consumes the embedding projection this way; layer 2
consumes the [own|peer] h-projection.  AllGathers are split in halves, with
the first half issued mid-recurrence.

Host assembles: cores 0..3 give forward-time logits for quarters 0..3.
"""
import numpy as np
import ml_dtypes

import concourse.bacc as bacc
import concourse.bass as bass
import concourse.mybir as mybir
import concourse.tile as tile
from concourse.bass_utils import run_bass_kernel_spmd

F32 = mybir.dt.float32
BF16 = mybir.dt.bfloat16
I32 = mybir.dt.int32
AF = mybir.ActivationFunctionType
BF16_NP = ml_dtypes.bfloat16

B, V, E, H, TAGS = 64, 50000, 512, 512, 50
NCORES = 8
BQ = B // 4  # 16: batch rows per core
_DONE = object()


def _build(T, passes=1):
    G = T * BQ // 128          # embedding row-groups of 128
    N16 = T * BQ               # history cols per h-slice
    CS = min(256, N16)         # projection chunk cols
    NCH = N16 // CS
    TPC = CS // BQ             # timesteps covered by one projection chunk
    PUMP = max(1, TPC // 16)   # pump one proj block every PUMP steps
    nc = bacc.Bacc("TRN2", target_bir_lowering=False, debug=False,
                   num_devices=NCORES)

    emb_d = nc.dram_tensor("emb", [V, E], F32, kind="ExternalInput").ap()
    idx_d = nc.dram_tensor("idx", [128, G], I32, kind="ExternalInput").ap()
    idxh_d = nc.dram_tensor("idxh", [128, 4], I32, kind="ExternalInput").ap()
    w1_d = nc.dram_tensor("w1", [4, 128, 2048], BF16, kind="ExternalInput").ap()
    b1_d = nc.dram_tensor("b1", [1, 2048], BF16, kind="ExternalInput").ap()
    wm1_d = nc.dram_tensor("wm1", [4, 128, 2048], BF16, kind="ExternalInput").ap()
    w2_d = nc.dram_tensor("w2", [8, 128, 2048], BF16, kind="ExternalInput").ap()
    b2_d = nc.dram_tensor("b2", [1, 2048], BF16, kind="ExternalInput").ap()
    wm2_d = nc.dram_tensor("wm2", [4, 128, 2048], BF16, kind="ExternalInput").ap()
    fcw_d = nc.dram_tensor("fcw", [8, 128, 64], BF16, kind="ExternalInput").ap()
    fcb_d = nc.dram_tensor("fcb", [1, 64], BF16, kind="ExternalInput").ap()
    onesb_d = nc.dram_tensor("onesb", [1, 512], BF16, kind="ExternalInput").ap()
    id128_d = nc.dram_tensor("id128", [128, 128], F32, kind="ExternalInput").ap()
    idbf_d = nc.dram_tensor("idbf", [128, 128], BF16, kind="ExternalInput").ap()
    logits_d = nc.dram_tensor("logits", [N16, 64], F32,
                              kind="ExternalOutput").ap()

    with tile.TileContext(nc) as tc:
        with tc.tile_pool(name="pconst", bufs=1) as pconst, \
             tc.tile_pool(name="pdram", bufs=1, space="DRAM") as pdram, \
             tc.tile_pool(name="pcar", bufs=1) as pcar:
            onesb = pconst.tile([1, 512], BF16, name="onesb")
            id128 = pconst.tile([128, 128], F32, name="id128")
            idbf = pconst.tile([128, 128], BF16, name="idbf")
            idxs = pconst.tile([128, G], I32, name="idxs")
            idxh = pconst.tile([128, 4], I32, name="idxh")
            b1r = pconst.tile([1, 2048], BF16, name="b1r")
            b2r = pconst.tile([1, 2048], BF16, name="b2r")
            fcb = pconst.tile([1, 64], BF16, name="fcb")
            for sb, dr in ((onesb, onesb_d), (id128, id128_d),
                           (idbf, idbf_d), (idxs, idx_d), (idxh, idxh_d),
                           (b1r, b1_d), (b2r, b2_d), (fcb, fcb_d)):
                nc.sync.dma_start(sb[:], dr[:])

            # hist-exchange pieces: cols [0,a) are written LAST in the
            # recurrence (they hold high-tr reversed time), so the piece
            # over [0,a) is gathered last; splitting lets the early pieces'
            # collectives run under the recurrence.
            bounds = sorted({0, max(BQ * 2, min(512, N16 // 4)),
                             N16 // 4, N16 // 2, N16})
            PIECES = list(zip(bounds[:-1], bounds[1:]))
            NP = len(PIECES)
            rev1 = [pdram.tile([4, 128, hi - lo], BF16, name=f"rev1_{i}")
                    for i, (lo, hi) in enumerate(PIECES)]
            rev2 = [pdram.tile([4, 128, hi - lo], BF16, name=f"rev2_{i}")
                    for i, (lo, hi) in enumerate(PIECES)]
            ag1 = [pdram.tile([1024, hi - lo], BF16, name=f"ag1_{i}")
                   for i, (lo, hi) in enumerate(PIECES)]
            ag2 = [pdram.tile([1024, hi - lo], BF16, name=f"ag2_{i}")
                   for i, (lo, hi) in enumerate(PIECES)]

            def mk_hooks(rev_dram, ag):
                hooks = {}
                for i, (lo, hi) in enumerate(PIECES):
                    if lo == 0:
                        continue
                    tt = T - lo // BQ
                    hooks.setdefault(tt, []).append(i)
                return {tt: (lambda pcs=pcs: [ag_piece(rev_dram, ag, i)
                                              for i in sorted(pcs,
                                                              reverse=True)])
                        for tt, pcs in hooks.items()}

            def epilogue(rev_dram, ag):
                for i in reversed(range(1, NP)):
                    xpp_piece(ag, i)
                ag_piece(rev_dram, ag, 0)
                xpp_piece(ag, 0)
            AG_GROUPS = [[0, 4], [1, 5], [2, 6], [3, 7]]

            # cross-phase SBUF: peer features (reused: z2-proj then FC) and
            # compact per-k own history (reused: z2-proj then FC)
            xpp = [pcar.tile([128, N16], BF16, name=f"xpp_{k}")
                   for k in range(4)]
            hk = []  # created lazily after L1 (SBUF freed by then)

            def ag_piece(rev_dram, ag, i):
                nc.gpsimd.collective_compute(
                    "AllGather", mybir.AluOpType.bypass,
                    replica_groups=AG_GROUPS,
                    ins=[rev_dram[i][:].rearrange("k p t -> (k p) t")],
                    outs=[ag[i][:]])

            def xpp_piece(ag, i):
                lo, hi = PIECES[i]
                for k in range(4):
                    nc.gpsimd.indirect_dma_start(
                        out=xpp[k][:, lo:hi], out_offset=None, in_=ag[i][:],
                        in_offset=bass.IndirectOffsetOnAxis(
                            ap=idxh[:, k:k + 1], axis=0))

            # one projection chunk: 16 gate-blocks of [128, CS] into zbuf;
            # yields after each block so the caller can interleave.
            def proj_chunk(n, zbuf, wsb, bsb, nk, rhs_fn, pspool,
                           prefix=None):
                if prefix is not None:
                    yield from prefix(n)
                for Bk in range(16):
                    pg = pspool.tile([128, CS], F32, tag="pgz", bufs=2)
                    nc.tensor.matmul(pg[:],
                                     lhsT=bsb[:, 128 * Bk:128 * Bk + 128],
                                     rhs=onesb[:, 0:CS],
                                     start=True, stop=False)
                    for k in range(nk):
                        nc.tensor.matmul(
                            pg[:], lhsT=wsb[k][:, 128 * Bk:128 * Bk + 128],
                            rhs=rhs_fn(k, n),
                            start=False, stop=(k == nk - 1))
                    dst = (zbuf[:].rearrange("p (tl c) -> p tl c", c=256)
                           [:, :, 16 * Bk:16 * Bk + 16])
                    src = pg[:].rearrange("p (tl b) -> p tl b", b=BQ)
                    nc.vector.tensor_copy(dst, src)
                    yield

            # one LSTM layer: T recurrence steps consuming zbuf chunks that
            # are produced by interleaved projection pumping.
            def lstm_layer(pR, psR, mk_chunk, wm_sb, hist, rev_dram, hooks):
                nc.vector.memset(hist[:, 0:64], 0.0)
                zbufs = {}

                def start_chunk(n):
                    if n >= NCH:
                        return None
                    zb = pR.tile([128, TPC * 256], BF16, tag="zbuf", bufs=2)
                    zbufs[n] = zb
                    return mk_chunk(n, zb)

                g = start_chunk(0)
                for _ in g:
                    pass
                pending = start_chunk(1)

                # scan-fused cell state: O[t%2] holds (junk, c) interleaved
                # at (even, odd) cols; ig products are written into the
                # (shifted) even cols of the previous O so ONE
                # tensor_tensor_scan computes c = f*c_prev + i*g.
                # F holds (0, f) interleaved; its even cols stay zero.
                Ot = [pR.tile([128, 132], F32, name=f"Osc{i}")
                      for i in range(2)]
                # FO: (0,f) interleaved in cols 0:128, (junk,o) in 128:256
                Ft = [pR.tile([128, 256], F32, name=f"Fsc{i}")
                      for i in range(2)]
                for i in range(2):
                    nc.vector.memset(Ot[i][:], 0.0)
                    nc.vector.memset(Ft[i][:], 0.0)

                def odd(ap, n=128):
                    return ap[:, 0:n].rearrange(
                        "p (e two) -> p e two", two=2)[:, :, 1]

                for t in range(T):
                    n, tl = divmod(t, TPC)
                    zs = zbufs[n][:, 256 * tl:256 * (tl + 1)]
                    # one psum tile per gate (bufs=1: the WAR against the
                    # previous step's activation read is long satisfied)
                    pgs = [psR.tile([128, 64], F32, tag=f"pg{gp}",
                                    bufs=1, name=f"pg{gp}")
                           for gp in range(4)]
                    h_prev = hist[:, 64 * t:64 * t + 64]
                    O_prev, O_cur = Ot[(t + 1) % 2], Ot[t % 2]
                    F = Ft[t % 2]

                    def mm_group(gp):
                        pg = pgs[gp]
                        nc.tensor.matmul(
                            pg[:], lhsT=idbf[:],
                            rhs=zs[:, 64 * gp:64 * (gp + 1)],
                            start=True, stop=False, skip_group_check=True)
                        for Bk in range(4 * gp, 4 * gp + 4):
                            for k in range(4):
                                nc.tensor.matmul(
                                    pg[:, 16 * (Bk % 4):16 * (Bk % 4) + 16],
                                    lhsT=wm_sb[k][:, 128 * Bk:128 * Bk + 128],
                                    rhs=h_prev[:, 16 * k:16 * k + 16],
                                    start=False, stop=(k == 3),
                                    skip_group_check=True)
                    tg = pR.tile([128, 64], F32, tag="tg", bufs=2)
                    s_i = pR.tile([128, 64], F32, tag="si", bufs=2)
                    s_o = pR.tile([128, 64], F32, tag="so", bufs=2)
                    mm_group(0)                                   # g
                    nc.scalar.activation(tg[:], pgs[0][:], AF.Tanh)
                    mm_group(1)                                   # i
                    nc.scalar.activation(s_i[:], pgs[1][:], AF.Sigmoid)
                    mm_group(2)                                   # f
                    mm_group(3)                                   # o
                    # ig -> shifted even cols of O_prev (off critical path)
                    nc.vector.tensor_mul(
                        O_prev[:, 2:130].rearrange(
                            "p (e two) -> p e two", two=2)[:, :, 0],
                        s_i[:], tg[:])
                    # f -> odd cols of F (even cols stay 0)
                    nc.scalar.activation(odd(F[:]), pgs[2][:], AF.Sigmoid)
                    nc.scalar.activation(s_o[:], pgs[3][:], AF.Sigmoid)
                    # c = f*c_prev + ig, interleaved in one scan
                    nc.vector.tensor_tensor_scan(
                        O_cur[:, 0:128], F[:, 0:128], O_prev[:, 1:129],
                        0.0, mybir.AluOpType.mult, mybir.AluOpType.add)
                    tc_t = pR.tile([128, 64], F32, tag="tc", bufs=2)
                    nc.scalar.activation(tc_t[:], odd(O_cur[:]), AF.Tanh)
                    h_new = hist[:, 64 * (t + 1):64 * (t + 2)]
                    nc.vector.tensor_mul(h_new, s_o[:], tc_t[:])
                    tr = T - 1 - t
                    rh = next(i for i, (lo, hi) in enumerate(PIECES)
                              if BQ * tr < hi)
                    rc = BQ * tr - PIECES[rh][0]
                    nc.sync.dma_start(
                        rev_dram[rh][:, :, rc:rc + BQ]
                        .rearrange("k p b -> p k b"),
                        h_new.rearrange("p (k b) -> p k b", b=BQ))

                    if pending is not None and t % PUMP == 0:
                        if next(pending, _DONE) is _DONE:
                            pending = None
                    if tl == TPC - 1:
                        if pending is not None:
                            for _ in pending:
                                pass
                        pending = start_chunk(n + 2)
                    if t in hooks:
                        hooks[t]()


            for _pass in range(passes):
                # ======== L1: embed + z1 proj interleaved with recurrence ====
                with tc.tile_pool(name="pH1", bufs=1) as pH1:
                    hist1 = pH1.tile([128, (T + 1) * 64], BF16, name="hist1")
                    with tc.tile_pool(name="pL1", bufs=1) as pL1, \
                         tc.tile_pool(name="psT", bufs=2, space="PSUM") as psT, \
                         tc.tile_pool(name="psP", bufs=2, space="PSUM") as psP, \
                         tc.tile_pool(name="psR", bufs=1, space="PSUM") as psR:
                        w1sb = [pL1.tile([128, 2048], BF16, name=f"w1sb{k}")
                                for k in range(4)]
                        wm1sb = [pL1.tile([128, 2048], BF16, name=f"wm1sb{k}")
                                 for k in range(4)]
                        for k in range(4):
                            nc.sync.dma_start(w1sb[k][:], w1_d[k])
                            nc.sync.dma_start(wm1sb[k][:], wm1_d[k])
                        xT = [pL1.tile([128, G * 128], BF16, name=f"xT{k}")
                              for k in range(4)]
                        GPC = CS // 128  # embedding row-groups per chunk

                        def embed_groups(n):
                            for g in range(GPC * n, GPC * (n + 1)):
                                es = pL1.tile([128, 512], F32, tag="es",
                                              bufs=2)
                                nc.gpsimd.indirect_dma_start(
                                    out=es[:], out_offset=None, in_=emb_d[:],
                                    in_offset=bass.IndirectOffsetOnAxis(
                                        ap=idxs[:, g:g + 1], axis=0))
                                for k in range(4):
                                    pt = psT.tile([128, 128], F32, tag="ptA")
                                    nc.tensor.transpose(
                                        pt[:], es[:, 128 * k:128 * (k + 1)],
                                        id128[:])
                                    nc.vector.tensor_copy(
                                        xT[k][:, 128 * g:128 * (g + 1)],
                                        pt[:])
                                yield

                        def mk1(n, zb):
                            return proj_chunk(
                                n, zb, w1sb, b1r, 4,
                                lambda k, nn: xT[k][:, CS * nn:CS * (nn + 1)],
                                psP, prefix=embed_groups)

                        lstm_layer(pL1, psR, mk1, wm1sb, hist1, rev1,
                                   mk_hooks(rev1, ag1))

                    # compact own history for the z2 projection
                    if not hk:
                        hk.extend(pcar.tile([128, N16], BF16, name=f"hk_{k}")
                                  for k in range(4))
                    hv = hist1[:, 64:].rearrange(
                        "p (t k b) -> p k t b", k=4, b=BQ)
                    for k in range(4):
                        nc.vector.tensor_copy(
                            hk[k][:].rearrange("p (t b) -> p t b", b=BQ),
                            hv[:, k])
                epilogue(rev1, ag1)

                # ======== L2: z2 proj interleaved with recurrence ============
                with tc.tile_pool(name="pH2", bufs=1) as pH2:
                    hist2 = pH2.tile([128, (T + 1) * 64], BF16, name="hist2")
                    with tc.tile_pool(name="pL2", bufs=1) as pL2, \
                         tc.tile_pool(name="psP2", bufs=2, space="PSUM") as psP2, \
                         tc.tile_pool(name="psR2", bufs=1, space="PSUM") as psR2:
                        w2sb = [pL2.tile([128, 2048], BF16, name=f"w2sb{k}")
                                for k in range(8)]
                        wm2sb = [pL2.tile([128, 2048], BF16, name=f"wm2sb{k}")
                                 for k in range(4)]
                        for k in range(8):
                            nc.sync.dma_start(w2sb[k][:], w2_d[k])
                        for k in range(4):
                            nc.sync.dma_start(wm2sb[k][:], wm2_d[k])

                        def rhs2(k, n):
                            if k < 4:
                                return hk[k][:, CS * n:CS * (n + 1)]
                            return xpp[k - 4][:, CS * n:CS * (n + 1)]

                        def mk2(n, zb):
                            return proj_chunk(n, zb, w2sb, b2r, 8, rhs2,
                                              psP2)

                        lstm_layer(pL2, psR2, mk2, wm2sb, hist2, rev2,
                                   mk_hooks(rev2, ag2))

                    # compact hist2 into hk (reused) for the FC; high half
                    # first -- the FC consumes high-m chunks first
                    hv2 = hist2[:, 64:].rearrange(
                        "p (t k b) -> p k t b", k=4, b=BQ)
                    for k in range(4):
                        nc.vector.tensor_copy(
                            hk[k][:, N16 // 2:].rearrange(
                                "p (t b) -> p t b", b=BQ),
                            hv2[:, k, T // 2:, :])
                    for k in range(4):
                        nc.vector.tensor_copy(
                            hk[k][:, 0:N16 // 2].rearrange(
                                "p (t b) -> p t b", b=BQ),
                            hv2[:, k, 0:T // 2, :])
                epilogue(rev2, ag2)

                # ======== FC ================================================
                with tc.tile_pool(name="pE", bufs=1) as pE, \
                     tc.tile_pool(name="psE", bufs=4, space="PSUM") as psE:
                    fcw = [pE.tile([128, 64], BF16, name=f"fcwsb{k}")
                           for k in range(8)]
                    for k in range(8):
                        nc.sync.dma_start(fcw[k][:], fcw_d[k])
                    for m in reversed(range(N16 // 128)):
                        pg = psE.tile([128, 64], F32, tag="pgE")
                        nc.tensor.matmul(pg[:], lhsT=onesb[:, 0:128],
                                         rhs=fcb[:], start=True, stop=False)
                        for k in range(8):
                            st = (hk[k][:, 128 * m:128 * (m + 1)]
                                  if k < 4 else
                                  xpp[k - 4][:, 128 * m:128 * (m + 1)])
                            nc.tensor.matmul(
                                pg[:], lhsT=st, rhs=fcw[k][:, :],
                                start=False, stop=(k == 7))
                        lst = pE.tile([128, 64], F32, tag="lst", bufs=3)
                        nc.vector.tensor_copy(lst[:], pg[:])
                        nc.sync.dma_start(
                            logits_d[128 * m:128 * (m + 1), :], lst[:])

    nc.compile()
    return nc


# ---------------- host-side data prep ----------------

GPERM = [2, 0, 1, 3]  # device gate order: g, i, f, o (torch: i, f, g, o)


def _wT(w):
    # w: [2048(torch gates), D] -> [D//128, 128, 2048] lhsT blocks:
    # out[k][p, 512g' + 128j + q] = w[512*GPERM[g'] + 128j + q, 128k + p]
    D = w.shape[1]
    w5 = w.reshape(4, 4, 128, D // 128, 128)[GPERM]   # [g', j, q, k, p]
    return np.ascontiguousarray(np.transpose(w5, (3, 4, 0, 1, 2))
                                .reshape(D // 128, 128, 2048))


def _bT(b):
    return b.reshape(4, 4, 128)[GPERM].reshape(1, 2048)


_CACHE = {}


def kernel(x, lengths, emb,
           Wih_f1, Whh_f1, bih_f1, bhh_f1,
           Wih_b1, Whh_b1, bih_b1, bhh_b1,
           Wih_f2, Whh_f2, bih_f2, bhh_f2,
           Wih_b2, Whh_b2, bih_b2, bhh_b2,
           fc_W, fc_b, _T=None, _passes=1):
    x = np.asarray(x)
    T = x.shape[1] if _T is None else _T
    G = T * BQ // 128

    key = (T, _passes)
    if key not in _CACHE:
        _CACHE[key] = _build(T, _passes)
    nc = _CACHE[key]

    emb = np.asarray(emb, np.float32)
    f32 = lambda a: np.asarray(a, np.float32)
    layers = {
        0: (f32(Wih_f1), f32(Whh_f1), f32(bih_f1) + f32(bhh_f1),
            f32(Wih_f2), f32(Whh_f2), f32(bih_f2) + f32(bhh_f2)),
        1: (f32(Wih_b1), f32(Whh_b1), f32(bih_b1) + f32(bhh_b1),
            f32(Wih_b2), f32(Whh_b2), f32(bih_b2) + f32(bhh_b2)),
    }
    fc_W = f32(fc_W)
    fc_b = f32(fc_b)

    common = {
        "emb": emb,
        "onesb": np.ones((1, 512), BF16_NP),
        "id128": np.eye(128, dtype=np.float32),
        "idbf": np.eye(128, dtype=np.float32).astype(BF16_NP),
        "fcb": np.pad(fc_b, (0, 14)).reshape(1, 64).astype(BF16_NP),
    }

    in_maps = []
    for i in range(NCORES):
        d, q = i // 4, i % 4
        wih1, whh1, bsum1, wih2, whh2, bsum2 = layers[d]
        xq = np.asarray(x[BQ * q:BQ * (q + 1), :T], np.int32)
        if d == 1:
            xq = xq[:, ::-1]
        # idx[p, g]: row r = 128g + p = 16t + b -> token xq[b, t]
        rr = np.arange(T * BQ)
        tt, bb = rr // BQ, rr % BQ
        idx_np = xq[bb, tt].reshape(G, 128).T.astype(np.int32).copy()
        # peer block in pairwise AG output: [fwd(512), bwd(512)]
        peer0 = 512 * (1 - d)
        pp = np.arange(128)
        idxh_np = (peer0 + 128 * np.arange(4)[None, :] + pp[:, None]
                   ).astype(np.int32)
        # layer-2 input feature order: own direction first, then peer
        own_sl = slice(512 * d, 512 * (d + 1))
        peer_sl = slice(512 * (1 - d), 512 * (2 - d))
        w2eff = np.concatenate([wih2[:, own_sl], wih2[:, peer_sl]], axis=1)
        fceff = np.concatenate([fc_W[:, own_sl], fc_W[:, peer_sl]], axis=1)
        fcmov = np.zeros((8, 128, 64), BF16_NP)
        for k in range(8):
            fcmov[k, :, :TAGS] = fceff[:, 128 * k:128 * (k + 1)].T
        in_maps.append(dict(
            common,
            idx=idx_np,
            idxh=idxh_np,
            w1=_wT(wih1).astype(BF16_NP),
            b1=_bT(bsum1).astype(BF16_NP),
            wm1=_wT(whh1).astype(BF16_NP),
            w2=_wT(w2eff).astype(BF16_NP),
            b2=_bT(bsum2).astype(BF16_NP),
            wm2=_wT(whh2).astype(BF16_NP),
            fcw=fcmov,
        ))

    res = run_bass_kernel_spmd(nc, in_maps, core_ids=list(range(NCORES)))

    out = np.zeros((B, T, TAGS), np.float32)
    for q in range(4):
        lg = res.results[q]["logits"][:, :TAGS]
        out[BQ * q:BQ * (q + 1)] = lg.reshape(T, BQ, TAGS).transpose(1, 0, 2)
    return out
